# revision 1
# baseline (speedup 1.0000x reference)
"""Trainium2 Bass kernel for batched low-rank (rank-64) KV-cache reconstruction.

Problem: for each of 64 matrices X [2048,128] (f32), compute the rank-64
truncated-SVD reconstruction  X_r = U_r diag(s_r) V_r^T.

Algorithm (device-side, per matrix; 8 matrices per NeuronCore, 8 cores):
  G = X^T X                               [128,128]
  Since rank = 64 = D/2, X_r = X P where P projects onto the span of the
  top-64 eigenvectors of G.  P = (I + sign(G - mu I))/2 for any mu strictly
  between eigenvalues lambda_64 and lambda_65 of G.
  mu is found by a fixed ladder of probes: each probe runs a short
  matmul-only polynomial sign iteration on (G - mu I)/R and takes the trace
  t ~= (#eigs above mu) - (#eigs below mu); mu is nudged by clamp(g*t, +-c).
  The final sign iteration (16 cubic steps) gives P, and the output is
  emitted transposed as P @ X^T (host transposes back).

All heavy ops are 128x128(x512) f32 matmuls on the PE array; per-step
elementwise combines run on DVE/ACT.
"""

from contextlib import ExitStack

import numpy as np

import concourse.bass as bass
import concourse.tile as tile
from concourse import bacc, mybir
from concourse.bass_utils import run_bass_kernel_spmd
from concourse.masks import make_identity

F32 = mybir.dt.float32
BF16 = mybir.dt.bfloat16
ts = bass.ts

N_CORES = 8
M_PER_CORE = 8          # matrices per core
S, D = 2048, 128        # matrix shape
NT = S // 128           # 16 row tiles

R = 1000.0              # spectral normalization for (G - mu I)/R
SEED_OFFSET = -41.4     # mu0 = trace(G)/128 + SEED_OFFSET

# probe ladder: (cubic coeff list, gain, clamp)
_P = lambda k: [2.0] * k
_PP = lambda k: [2.0] * k + [1.5, 1.5]
STAGES = [
    (_P(4), 8.0, 60.0),
    (_P(5), 8.0, 30.0),
    (_P(7), 7.0, 12.0),
    (_PP(9), 5.0, 5.0),
    (_PP(10), 3.0, 2.5),
    (_PP(11), 2.5, 1.2),
    (_PP(12), 1.2, 0.55),
    (_PP(12), 0.8, 0.28),
    (_PP(13), 0.5, 0.14),
]
FINAL = [2.0] * 12 + [1.5] * 4
DEBUG_DUMP = False


def _ns_step(nc, ps_pool, cpool, B, aI, a, m, dt=F32, pair=False):
    """One cubic step B <- B @ (a*I - (a-1) B^2). B is an SBUF [128,128] tile.

    pair=True computes ps3 = B^T C + (B^T C)^T (bit-exactly symmetric: the two
    accumulation groups mirror each other entrywise and IEEE add commutes),
    then B <- 0.5*ps3.  This kills the skew-noise amplification that otherwise
    blows up low-precision iterates (one bf16-ulp of asymmetry grows ~3x/step).
    """
    ps2 = ps_pool.tile([128, 128], F32, tag="ps2", name="ps2", bufs=4)
    nc.tensor.matmul(ps2[:], B[:], B[:], start=True, stop=True)
    C = cpool.tile([128, 128], dt, tag=f"c_{m % 2}", name=f"c_{m % 2}")
    nc.vector.scalar_tensor_tensor(
        out=C[:], in0=ps2[:], scalar=-(a - 1.0), in1=aI[:],
        op0=mybir.AluOpType.mult, op1=mybir.AluOpType.add,
    )
    ps3 = ps_pool.tile([128, 128], F32, tag="ps3", name="ps3", bufs=3)
    if pair:
        nc.tensor.matmul(ps3[:], B[:], C[:], start=True, stop=False)
        nc.tensor.matmul(ps3[:], C[:], B[:], start=False, stop=True)
        nc.vector.tensor_scalar_mul(B[:], ps3[:], 0.5)
    else:
        nc.tensor.matmul(ps3[:], B[:], C[:], start=True, stop=True)
        nc.any.tensor_copy(B[:], ps3[:])


def _shift_normalize(nc, cpool, B, G, iR, MU, m):
    """B = (G - mu I)/R = G*(1/R) - mu*(I/R);  MU[:, m] broadcasts mu_m."""
    MuI = cpool.tile([128, 128], F32, tag=f"mui_{m % 2}", name=f"mui_{m % 2}")
    nc.vector.tensor_scalar_mul(MuI[:], iR[:], MU[:, m : m + 1])
    nc.vector.scalar_tensor_tensor(
        out=B[:], in0=G[:], scalar=1.0 / R, in1=MuI[:],
        op0=mybir.AluOpType.mult, op1=mybir.AluOpType.subtract,
    )


def _trace_to_pack(nc, cpool, B, ident, trpack, m):
    """trpack[:, m] = per-partition sum of (B * I) -> column of diag entries."""
    scr = cpool.tile([128, 128], F32, tag=f"scr_{m % 2}", name=f"scr_{m % 2}")
    nc.vector.tensor_tensor(scr[:], B[:], ident[:], op=mybir.AluOpType.mult)
    nc.vector.tensor_reduce(
        trpack[:, m : m + 1], scr[:], axis=mybir.AxisListType.X,
        op=mybir.AluOpType.add,
    )


def _kernel_body(tc, nc, ctx, x, y):
    dbg = None
    if DEBUG_DUMP:
        dbg = nc.dram_tensor("dbg", [2, 16, M_PER_CORE], F32, kind="ExternalOutput").ap()
    consts = ctx.enter_context(tc.tile_pool(name="consts", bufs=1))
    gpool = ctx.enter_context(tc.tile_pool(name="gpool", bufs=1))
    bpool = ctx.enter_context(tc.tile_pool(name="bpool", bufs=1))
    xtpool = ctx.enter_context(tc.tile_pool(name="xtpool", bufs=1))
    xpool = ctx.enter_context(tc.tile_pool(name="xpool", bufs=4))
    cpool = ctx.enter_context(tc.tile_pool(name="cpool", bufs=3))
    opool = ctx.enter_context(tc.tile_pool(name="opool", bufs=3))
    ps = ctx.enter_context(tc.tile_pool(name="ps", bufs=2, space="PSUM"))

    # constants
    ident = consts.tile([128, 128], F32, tag="ident", name="ident")
    make_identity(nc, ident[:])
    iR = consts.tile([128, 128], F32, tag="iR", name="iR")
    nc.vector.tensor_scalar_mul(iR[:], ident[:], 1.0 / R)
    i20 = consts.tile([128, 128], F32, tag="i20", name="i20")
    nc.vector.tensor_scalar_mul(i20[:], ident[:], 2.0)
    i15 = consts.tile([128, 128], F32, tag="i15", name="i15")
    nc.vector.tensor_scalar_mul(i15[:], ident[:], 1.5)
    i05 = consts.tile([128, 128], F32, tag="i05", name="i05")
    nc.vector.tensor_scalar_mul(i05[:], ident[:], 0.5)
    ones_col = consts.tile([128, 1], F32, tag="ones_col", name="ones_col")
    nc.vector.memset(ones_col[:], 1.0)
    ones_row = consts.tile([1, 128], F32, tag="ones_row", name="ones_row")
    nc.vector.memset(ones_row[:], 1.0)

    trpack = consts.tile([128, M_PER_CORE], F32, tag="trpack", name="trpack")
    mu_row = consts.tile([1, M_PER_CORE], F32, tag="mu_row", name="mu_row")
    step_row = consts.tile([1, M_PER_CORE], F32, tag="step_row", name="step_row")

    G = [gpool.tile([128, 128], F32, tag=f"G{m}", name=f"G{m}") for m in range(M_PER_CORE)]
    B = [bpool.tile([128, 128], F32, tag=f"B{m}", name=f"B{m}") for m in range(M_PER_CORE)]
    XT = [xtpool.tile([128, S], F32, tag=f"XT{m}", name=f"XT{m}") for m in range(M_PER_CORE)]

    # ---- Phase A: load, Gram, transpose, trace seed ----
    for m in range(M_PER_CORE):
        psG = ps.tile([128, 128], F32, tag="ps2", name="psG", bufs=4)
        for t in range(NT):
            xt = xpool.tile([128, 128], F32, tag=f"x_{t % 4}", name=f"x_{t % 4}")
            nc.sync.dma_start(xt[:], x[m, ts(t, 128), :])
            nc.tensor.matmul(
                psG[:], xt[:], xt[:], start=(t == 0), stop=(t == NT - 1)
            )
            psT = ps.tile([128, 128], F32, tag="ps3", name="psT", bufs=3)
            nc.tensor.transpose(psT[:], xt[:], ident[:])
            nc.any.tensor_copy(XT[m][:, ts(t, 128)], psT[:])
        nc.any.tensor_copy(G[m][:], psG[:])
        _trace_to_pack(nc, cpool, G[m], ident, trpack, m)

    ps_tr = ps.tile([1, M_PER_CORE], F32, tag="pstr", name="pstr", bufs=1)
    nc.tensor.matmul(ps_tr[:], ones_col[:], trpack[:], start=True, stop=True)
    nc.vector.tensor_scalar(
        out=mu_row[:], in0=ps_tr[:], scalar1=1.0 / 128.0, scalar2=SEED_OFFSET,
        op0=mybir.AluOpType.mult, op1=mybir.AluOpType.add,
    )

    # ---- Phase B: probe ladder ----
    for probe_i, (coeffs, gain, clamp) in enumerate(STAGES):
        ps_bc = ps.tile([128, M_PER_CORE], F32, tag="pstr", name="pstr", bufs=1)
        nc.tensor.matmul(ps_bc[:], ones_row[:], mu_row[:], start=True, stop=True)
        MU = cpool.tile([128, M_PER_CORE], F32, tag="mu_bcast", name="mu_bcast")
        nc.any.tensor_copy(MU[:], ps_bc[:])
        for m in range(M_PER_CORE):
            _shift_normalize(nc, cpool, B[m], G[m], iR, MU, m)
        for a in coeffs:
            for m in range(M_PER_CORE):
                _ns_step(nc, ps, cpool, B[m], i20 if a == 2.0 else i15, a, m)
        for m in range(M_PER_CORE):
            _trace_to_pack(nc, cpool, B[m], ident, trpack, m)
        ps_tr = ps.tile([1, M_PER_CORE], F32, tag="pstr", name="pstr", bufs=1)
        nc.tensor.matmul(ps_tr[:], ones_col[:], trpack[:], start=True, stop=True)
        # step = clamp(gain * t, +-clamp); mu += step
        nc.vector.tensor_scalar(
            out=step_row[:], in0=ps_tr[:], scalar1=gain, scalar2=clamp,
            op0=mybir.AluOpType.mult, op1=mybir.AluOpType.min,
        )
        nc.vector.tensor_scalar_max(step_row[:], step_row[:], -clamp)
        if DEBUG_DUMP:
            dbg_mu = cpool.tile([1, M_PER_CORE], F32, tag=f"dbgmu{probe_i}", name="dbg_mu", bufs=1)
            nc.vector.tensor_copy(dbg_mu[:], mu_row[:])
            nc.sync.dma_start(dbg[0, probe_i, 0:M_PER_CORE], dbg_mu[0, :])
            dbg_t = cpool.tile([1, M_PER_CORE], F32, tag=f"dbgt{probe_i}", name="dbg_t", bufs=1)
            nc.vector.tensor_copy(dbg_t[:], step_row[:])
            nc.sync.dma_start(dbg[1, probe_i, 0:M_PER_CORE], dbg_t[0, :])
        nc.vector.tensor_tensor(
            mu_row[:], mu_row[:], step_row[:], op=mybir.AluOpType.add
        )

    # ---- Phase C: final sign -> P -> reconstruction (transposed out) ----
    ps_bc = ps.tile([128, M_PER_CORE], F32, tag="pstr", name="pstr", bufs=1)
    nc.tensor.matmul(ps_bc[:], ones_row[:], mu_row[:], start=True, stop=True)
    MU = cpool.tile([128, M_PER_CORE], F32, tag="mu_bcast", name="mu_bcast")
    nc.any.tensor_copy(MU[:], ps_bc[:])
    for m in range(M_PER_CORE):
        _shift_normalize(nc, cpool, B[m], G[m], iR, MU, m)
    for si, a in enumerate(FINAL):
        for m in range(M_PER_CORE):
            _ns_step(nc, ps, cpool, B[m], i20 if a == 2.0 else i15, a, m)
        if si in (3, 7, 11):
            # symmetrize: the lhsT-transposed matmul form amplifies
            # skew-symmetric f32 noise (~3x/step near convergence)
            for m in range(M_PER_CORE):
                psT = ps.tile([128, 128], F32, tag="ps2", name="psSym", bufs=4)
                nc.tensor.transpose(psT[:], B[m][:], ident[:])
                nc.vector.scalar_tensor_tensor(
                    out=B[m][:], in0=psT[:], scalar=1.0, in1=B[m][:],
                    op0=mybir.AluOpType.mult, op1=mybir.AluOpType.add,
                )
                nc.vector.tensor_scalar_mul(B[m][:], B[m][:], 0.5)
    for m in range(M_PER_CORE):
        # P = 0.5*B + 0.5*I  (reuse B tile as P)
        nc.vector.scalar_tensor_tensor(
            out=B[m][:], in0=B[m][:], scalar=0.5, in1=i05[:],
            op0=mybir.AluOpType.mult, op1=mybir.AluOpType.add,
        )
        for ch in range(S // 512):
            psO = ps.tile([128, 512], F32, tag="ps2", name="pso", bufs=4)
            nc.tensor.matmul(
                psO[:], B[m][:], XT[m][:, ts(ch, 512)], start=True, stop=True
            )
            osb = opool.tile([128, 512], F32, tag=f"o_{ch % 2}", name=f"o_{ch % 2}")
            nc.any.tensor_copy(osb[:], psO[:])
            nc.sync.dma_start(y[m, :, ts(ch, 512)], osb[:])


_NC_CACHE = {}


def _build_program():
    if "nc" in _NC_CACHE:
        return _NC_CACHE["nc"]
    nc = bacc.Bacc(
        "TRN2",
        target_bir_lowering=False,
        debug=False,
        enable_asserts=True,
        num_devices=N_CORES,
    )
    x = nc.dram_tensor("x", [M_PER_CORE, S, D], F32, kind="ExternalInput").ap()
    y = nc.dram_tensor("y", [M_PER_CORE, D, S], F32, kind="ExternalOutput").ap()
    with tile.TileContext(nc) as tc:
        with ExitStack() as ctx:
            _kernel_body(tc, nc, ctx, x, y)
    nc.compile()
    _NC_CACHE["nc"] = nc
    return nc


def kernel(kv_cache, rank, **_ignored):
    kv = np.asarray(kv_cache)
    assert kv.shape == (4, 16, S, D), kv.shape
    assert int(rank) == 64, rank
    orig_dtype = kv.dtype
    xs = np.ascontiguousarray(kv.reshape(-1, S, D).astype(np.float32))

    nc = _build_program()
    in_maps = [
        {"x": xs[i * M_PER_CORE : (i + 1) * M_PER_CORE]} for i in range(N_CORES)
    ]
    res = run_bass_kernel_spmd(nc, in_maps, list(range(N_CORES)))
    outs = [np.asarray(res.results[i]["y"]) for i in range(N_CORES)]
    yt = np.concatenate(outs, axis=0)          # [64, 128, 2048]
    out = yt.transpose(0, 2, 1).reshape(4, 16, S, D)
    return out.astype(orig_dtype, copy=False)


if __name__ == "__main__":
    rng = np.random.default_rng(0)
    kv = rng.standard_normal((4, 16, S, D)).astype(np.float32)
    out = kernel(kv_cache=kv, rank=64)
    print("kernel ran, out", out.shape, out.dtype)



# revision 7
# speedup vs baseline: 2.5733x; 2.5733x over previous
"""Trainium2 Bass kernel for batched low-rank (rank-64) KV-cache reconstruction.

Problem: for each of 64 matrices X [2048,128] (f32), compute the rank-64
truncated-SVD reconstruction X_r = U_r diag(s_r) V_r^T = X P where P projects
onto the top-64 eigenspace of G = X^T X.

Per NeuronCore (8 matrices), mixed fp16/fp32:
  G = Xh^T Xh  (Xh = fp16(X), f32 PSUM accumulate)
  mu ladder: 4 probe stages of cubic soft-sign iterations (a=2 growth +
    a=1.5 landing steps, all fp16) reading trace(B_k) as an eigen-count to
    root-find mu between lambda_64 and lambda_65.
  final: 12-step sign iteration (optimized cubic coefficient schedule,
    2 fp32 head steps then fp16, periodic symmetric "pair" steps),
    P = (sign + I)/2 emitted by the last write-back.
  recon: Y^T = P @ Xh^T via fp16 matmuls (XT loaded by DMA-transpose XBAR).

The cubic step is computed as Ct = I - (b/a) B^2; B' = a * (B @ Ct), folding
the `a` coefficient into the PSUM->SBUF write-back scale, so a single
identity constant serves every step. Probes read trace(B@C) = sum(B * Ct) * a
(B, Ct symmetric) skipping the last matmul.  Vector ops are batched over
groups of 4 matrices ([128,512] tiles); the two groups run independent
mu-update chains.
"""

from contextlib import ExitStack

import numpy as np

import concourse.bass as bass
import concourse.tile as tile
from concourse import bacc, mybir
from concourse.bass_utils import run_bass_kernel_spmd
from concourse.masks import make_identity

F32 = mybir.dt.float32
F16 = mybir.dt.float16
AF = mybir.ActivationFunctionType
OP = mybir.AluOpType
ts = bass.ts

N_CORES = 8
M_PER_CORE = 8
NG = 2               # matrix groups per core
GM = 4               # matrices per group
S, D = 2048, 128
NT = S // 128        # 16 row chunks

R = 1250.0
OFF = -41.37

GROW = (2.0, 1.0)
LAND = (1.5, 0.5)
# ladder stages: (n_growth, n_land, gain, clamp); all fp16
LADDER = [
    (4, 1, 7.75, 28.0),
    (6, 1, 6.0, 8.0),
    (7, 2, 2.8, 4.0),
    (10, 2, 0.55, 1.0),
]
# final sign schedule (a, b); steps 0-1 run in fp32
FINAL = [
    (3.25, 3.10818),
    (2.59, 1.575008),
    (2.59, 1.571647),
    (2.59, 1.573557),
    (2.58, 1.553639),
    (2.56, 1.519041),
    (2.52, 1.450415),
    (2.4, 1.250828),
    (2.14, 0.887286),
    (1.78, 0.510408),
    (1.206349, 0.259768),
    (1.5, 0.5),
]
FINAL_F32_HEAD = 2
DEBUG_MU = False


def _probe_coeffs(n_grow, n_land):
    return [GROW] * n_grow + [LAND] * n_land


def _pair_steps(dts):
    """Every 3rd fp16 step runs as a symmetric pair step."""
    out = set()
    n = 0
    for i, is16 in enumerate(dts):
        if is16:
            n += 1
            if n % 3 == 0:
                out.add(i)
    return out


class Ctx:
    pass


def _build_consts(tc, nc, ctx, c):
    consts = ctx.enter_context(tc.tile_pool(name="consts", bufs=1))
    c.ident = consts.tile([128, 128], F32, tag="ident", name="ident")
    make_identity(nc, c.ident[:])
    c.iOverR = consts.tile([128, 128], F32, tag="iOverR", name="iOverR")
    nc.vector.tensor_scalar_mul(c.iOverR[:], c.ident[:], 1.0 / R)
    c.I_rep = consts.tile([128, 512], F32, tag="I_rep", name="I_rep")
    for b in range(GM):
        nc.gpsimd.tensor_copy(c.I_rep[:, ts(b, 128)], c.ident[:])
    c.I05_rep = consts.tile([128, 512], F32, tag="I05_rep", name="I05_rep")
    nc.vector.tensor_scalar_mul(c.I05_rep[:], c.I_rep[:], 0.5)
    c.ones_col = consts.tile([128, 1], F32, tag="ones_col", name="ones_col")
    nc.vector.memset(c.ones_col[:], 1.0)
    c.ones_row = consts.tile([1, 128], F32, tag="ones_row", name="ones_row")
    nc.vector.memset(c.ones_row[:], 1.0)
    # per-group mu state
    c.trpack = [consts.tile([128, GM], F32, tag=f"trpack{g}", name=f"trpack{g}") for g in range(NG)]
    c.mu_row = [consts.tile([1, GM], F32, tag=f"mu_row{g}", name=f"mu_row{g}") for g in range(NG)]
    c.step_row = [consts.tile([1, GM], F32, tag=f"step{g}", name=f"step{g}") for g in range(NG)]
    c.MU = [consts.tile([128, GM], F32, tag=f"MU{g}", name=f"MU{g}") for g in range(NG)]


def _trace_cols(nc, c, pools, g, in0_ap, in1_ap, scale_to_f32=True):
    """trpack[g][:, :] = per-partition sums of (in0 * in1) per 128-block."""
    scr = pools.scr.tile([128, 512], F32, tag=f"scr{g}", name=f"scr{g}", bufs=2)
    nc.vector.tensor_tensor(scr[:], in0_ap, in1_ap, op=OP.mult)
    nc.vector.tensor_reduce(
        c.trpack[g][:, :], scr[:].rearrange("p (b d) -> p b d", d=128),
        axis=mybir.AxisListType.X, op=OP.add,
    )


def _mu_broadcast(nc, c, pools, g):
    ps_mu = pools.pmisc.tile([128, GM], F32, tag=f"psmu{g}", name=f"psmu{g}", bufs=1)
    nc.tensor.matmul(ps_mu[:], c.ones_row[:], c.mu_row[g][:], start=True, stop=True)
    nc.scalar.copy(c.MU[g][:], ps_mu[:])


def _mu_from_trace(nc, c, pools, g, scale, offset=None, clamp=None):
    """seed: mu = t*scale + offset;  update: mu += clip(t*scale, +-clamp)."""
    ps_tr = pools.pmisc.tile([1, GM], F32, tag=f"pstr{g}", name=f"pstr{g}", bufs=1)
    nc.tensor.matmul(ps_tr[:], c.ones_col[:], c.trpack[g][:], start=True, stop=True)
    if offset is not None:
        nc.vector.tensor_scalar(
            out=c.mu_row[g][:], in0=ps_tr[:], scalar1=scale, scalar2=offset,
            op0=OP.mult, op1=OP.add,
        )
    else:
        nc.vector.tensor_scalar(
            out=c.step_row[g][:], in0=ps_tr[:], scalar1=scale, scalar2=clamp,
            op0=OP.mult, op1=OP.min,
        )
        nc.vector.tensor_scalar_max(c.step_row[g][:], c.step_row[g][:], -clamp)
        nc.vector.tensor_tensor(
            c.mu_row[g][:], c.mu_row[g][:], c.step_row[g][:], op=OP.add
        )
    _mu_broadcast(nc, c, pools, g)


def _make_B0(nc, c, pools, g, B_dst, muI):
    """B0 = G/R - mu*I/R for each matrix in group g (batched STT + per-m MuI)."""
    for j in range(GM):
        nc.gpsimd.tensor_scalar_mul(
            muI[:, ts(j, 128)], c.iOverR[:], c.MU[g][:, j : j + 1]
        )
    nc.vector.scalar_tensor_tensor(
        out=B_dst[:], in0=pools.G[g][:], scalar=1.0 / R, in1=muI[:],
        op0=OP.mult, op1=OP.subtract,
    )


def _cubic_steps(nc, c, pools, g, B, coeffs, f16_mask, pair, wb_engines,
                 last_trace=False, last_P=None):
    """Run cubic steps on group-g batched iterate B ([128,512]).

    coeffs: list of (a,b); f16_mask[i]: step runs fp16 (C dtype + write-back);
    pair: set of pair-step indices; wb_engines cycles write-back engines.
    last_trace: final step computes trpack instead of ps3 (+no write-back).
    last_P: if set (AP), last write-back emits P = 0.5*ps3*a + 0.5*I.
    """
    k = len(coeffs)
    for i, (a, b) in enumerate(coeffs):
        dt = F16 if f16_mask[i] else F32
        ps2 = pools.p2.tile([128, 512], F32, tag=f"ps2g{g}", name=f"ps2g{g}", bufs=1)
        for j in range(GM):
            nc.tensor.matmul(
                ps2[:, ts(j, 128)], B[:, ts(j, 128)], B[:, ts(j, 128)],
                start=True, stop=True,
            )
        Ct = pools.ct.tile([128, 512], dt, tag=f"ct{g}_{dt}", name=f"ct{g}", bufs=2)
        nc.vector.scalar_tensor_tensor(
            out=Ct[:], in0=ps2[:], scalar=-(b / a), in1=c.I_rep[:],
            op0=OP.mult, op1=OP.add,
        )
        if last_trace and i == k - 1:
            _trace_cols(nc, c, pools, g, B[:], Ct[:])
            return
        ps3 = pools.p3.tile([128, 512], F32, tag=f"ps3g{g}", name=f"ps3g{g}", bufs=1)
        is_pair = i in pair
        for j in range(GM):
            nc.tensor.matmul(
                ps3[:, ts(j, 128)], B[:, ts(j, 128)], Ct[:, ts(j, 128)],
                start=True, stop=not is_pair,
            )
            if is_pair:
                # close each block's accumulation group before opening the
                # next: only one PSUM accumulation group may be open at a time
                nc.tensor.matmul(
                    ps3[:, ts(j, 128)], Ct[:, ts(j, 128)], B[:, ts(j, 128)],
                    start=False, stop=True, skip_group_check=True,
                )
        scale = a / 2.0 if is_pair else a
        if last_P is not None and i == k - 1:
            nc.vector.scalar_tensor_tensor(
                out=last_P, in0=ps3[:], scalar=0.5 * scale, in1=c.I05_rep[:],
                op0=OP.mult, op1=OP.add,
            )
            return
        eng = wb_engines[i % len(wb_engines)]
        if eng == "act":
            nc.scalar.activation(B[:], ps3[:], AF.Copy, scale=scale)
        else:
            nc.vector.tensor_scalar_mul(B[:], ps3[:], scale)


def _kernel_body(tc, nc, ctx, x, y, dbg=None):
    c = Ctx()
    _build_consts(tc, nc, ctx, c)

    pools = Ctx()
    xt_pool = ctx.enter_context(tc.tile_pool(name="xt", bufs=1))
    xp_pool = ctx.enter_context(tc.tile_pool(name="xp", bufs=1))
    gpool = ctx.enter_context(tc.tile_pool(name="G", bufs=1))
    bpool = ctx.enter_context(tc.tile_pool(name="B", bufs=1))
    pools.ct = ctx.enter_context(tc.tile_pool(name="ct", bufs=2))
    pools.scr = ctx.enter_context(tc.tile_pool(name="scr", bufs=2))
    mpool = ctx.enter_context(tc.tile_pool(name="muI", bufs=1))
    opool = ctx.enter_context(tc.tile_pool(name="osb", bufs=3))
    pools.p2 = ctx.enter_context(tc.tile_pool(name="p2", bufs=1, space="PSUM"))
    pools.p3 = ctx.enter_context(tc.tile_pool(name="p3", bufs=1, space="PSUM"))
    pools.pmisc = ctx.enter_context(tc.tile_pool(name="pmisc", bufs=1, space="PSUM"))

    XT = [xt_pool.tile([128, S], F16, tag=f"XT{m}", name=f"XT{m}") for m in range(M_PER_CORE)]
    XP = [xp_pool.tile([128, S], F16, tag=f"XP{m}", name=f"XP{m}", bufs=1)
          for m in range(M_PER_CORE)]
    pools.G = [gpool.tile([128, 512], F32, tag=f"G{g}", name=f"G{g}") for g in range(NG)]
    B16 = [bpool.tile([128, 512], F16, tag=f"B16_{g}", name=f"B16_{g}") for g in range(NG)]
    B32 = [bpool.tile([128, 512], F32, tag=f"B32_{g}", name=f"B32_{g}") for g in range(NG)]
    P16 = [bpool.tile([128, 512], F16, tag=f"P16_{g}", name=f"P16_{g}") for g in range(NG)]
    muI = [mpool.tile([128, 512], F32, tag=f"muI{g}", name=f"muI{g}") for g in range(NG)]

    # ---- Phase A: loads + Gram + seed ----
    # packed row layout: XP[m][p, t*128+d] = X[m][p*16+t, d]  (4KB runs)
    for m in range(M_PER_CORE):
        src = x[m].rearrange("(p t) d -> p t d", t=NT)
        dst = XP[m][:].rearrange("p (t d) -> p t d", d=128)
        eng = nc.sync if m % 2 == 0 else nc.scalar
        eng.dma_start(dst, src)
    for g in range(NG):
        psG = pools.p2.tile([128, 512], F32, tag=f"ps2g{g}", name=f"psG{g}", bufs=1)
        for j in range(GM):
            m = g * GM + j
            for t in range(NT):
                nc.tensor.matmul(
                    psG[:, ts(j, 128)], XP[m][:, ts(t, 128)], XP[m][:, ts(t, 128)],
                    start=(t == 0), stop=(t == NT - 1),
                )
        nc.scalar.copy(pools.G[g][:], psG[:])
        _trace_cols(nc, c, pools, g, pools.G[g][:], c.I_rep[:])
        _mu_from_trace(nc, c, pools, g, 1.0 / 128.0, offset=OFF)
    # XT via DMA transpose XBAR (fills during the ladder; needed at recon)
    for m in range(M_PER_CORE):
        nc.sync.dma_start(XT[m][:], x[m], transpose=True)

    # ---- Phase B: mu ladder (fp16) ----
    for si, (n_grow, n_land, gain, clamp) in enumerate(LADDER):
        coeffs = _probe_coeffs(n_grow, n_land)
        k = len(coeffs)
        f16_mask = [True] * k
        pair = _pair_steps(f16_mask)
        pair.discard(k - 1)
        for g in range(NG):
            _make_B0(nc, c, pools, g, B16[g], muI[g])
            _cubic_steps(nc, c, pools, g, B16[g][:], coeffs, f16_mask, pair,
                         ["act" if g == 0 else "dve"], last_trace=True)
            a_last = coeffs[-1][0]
            _mu_from_trace(nc, c, pools, g, gain * a_last, clamp=clamp)
            if DEBUG_MU and dbg is not None:
                dmu = pools.scr.tile([1, GM], F32, tag=f"dmu{g}", name="dmu", bufs=1)
                nc.vector.tensor_copy(dmu[:], c.mu_row[g][:])
                nc.sync.dma_start(dbg[si, g * GM : (g + 1) * GM], dmu[0, :])

    # ---- Phase C: final sign -> P ----
    k = len(FINAL)
    f16_mask = [i >= FINAL_F32_HEAD for i in range(k)]
    pair = _pair_steps(f16_mask)
    pair.discard(k - 1)
    for g in range(NG):
        _make_B0(nc, c, pools, g, B32[g], muI[g])
        # fp32 head steps operate on B32; write-back of the last head step
        # lands in B16 (cast); then fp16 steps continue on B16.
        head = FINAL[:FINAL_F32_HEAD]
        for i, (a, b) in enumerate(head):
            ps2 = pools.p2.tile([128, 512], F32, tag=f"ps2g{g}", name=f"ps2g{g}", bufs=1)
            for j in range(GM):
                nc.tensor.matmul(
                    ps2[:, ts(j, 128)], B32[g][:, ts(j, 128)], B32[g][:, ts(j, 128)],
                    start=True, stop=True,
                )
            Ct = pools.ct.tile([128, 512], F32, tag=f"ct{g}_f32", name=f"ct32{g}", bufs=2)
            nc.vector.scalar_tensor_tensor(
                out=Ct[:], in0=ps2[:], scalar=-(b / a), in1=c.I_rep[:],
                op0=OP.mult, op1=OP.add,
            )
            ps3 = pools.p3.tile([128, 512], F32, tag=f"ps3g{g}", name=f"ps3g{g}", bufs=1)
            for j in range(GM):
                nc.tensor.matmul(
                    ps3[:, ts(j, 128)], B32[g][:, ts(j, 128)], Ct[:, ts(j, 128)],
                    start=True, stop=True,
                )
            dst = B32[g][:] if i < FINAL_F32_HEAD - 1 else B16[g][:]
            nc.scalar.activation(dst, ps3[:], AF.Copy, scale=a)
        _cubic_steps(nc, c, pools, g, B16[g][:], FINAL[FINAL_F32_HEAD:],
                     [True] * (k - FINAL_F32_HEAD),
                     {i - FINAL_F32_HEAD for i in pair},
                     ["act" if g == 0 else "dve"], last_P=P16[g][:])

    # ---- Phase D: recon Y^T = P @ X^T ----
    for g in range(NG):
        for j in range(GM):
            m = g * GM + j
            for ch in range(S // 512):
                psO = pools.p3.tile([128, 512], F32, tag=f"ps3g{g}", name=f"psO{g}", bufs=1)
                nc.tensor.matmul(
                    psO[:], P16[g][:, ts(j, 128)], XT[m][:, ts(ch, 512)],
                    start=True, stop=True,
                )
                osb = opool.tile([128, 512], F16, tag=f"o{m % 2}", name=f"o{m % 2}", bufs=3)
                if ch % 2 == 0:
                    nc.scalar.copy(osb[:], psO[:])
                else:
                    nc.vector.tensor_copy(osb[:], psO[:])
                eng = nc.sync if m % 2 == 0 else nc.scalar
                eng.dma_start(y[m, :, ts(ch, 512)], osb[:])


_NC_CACHE = {}


def _build_program():
    if "nc" in _NC_CACHE:
        return _NC_CACHE["nc"]
    nc = bacc.Bacc(
        "TRN2",
        target_bir_lowering=False,
        debug=False,
        enable_asserts=True,
        num_devices=N_CORES,
    )
    x = nc.dram_tensor("x", [M_PER_CORE, S, D], F16, kind="ExternalInput").ap()
    y = nc.dram_tensor("y", [M_PER_CORE, D, S], F16, kind="ExternalOutput").ap()
    dbg = None
    if DEBUG_MU:
        dbg = nc.dram_tensor("dbg", [len(LADDER), M_PER_CORE], F32, kind="ExternalOutput").ap()
    with tile.TileContext(nc) as tc:
        with ExitStack() as ctx:
            _kernel_body(tc, nc, ctx, x, y, dbg)
    nc.compile()
    _NC_CACHE["nc"] = nc
    return nc


def kernel(kv_cache, rank, **_ignored):
    kv = np.asarray(kv_cache)
    assert kv.shape == (4, 16, S, D), kv.shape
    assert int(rank) == 64, rank
    orig_dtype = kv.dtype
    xs = np.ascontiguousarray(kv.reshape(-1, S, D)).astype(np.float16)

    nc = _build_program()
    in_maps = [
        {"x": xs[i * M_PER_CORE : (i + 1) * M_PER_CORE]} for i in range(N_CORES)
    ]
    res = run_bass_kernel_spmd(nc, in_maps, list(range(N_CORES)))
    outs = [np.asarray(res.results[i]["y"]) for i in range(N_CORES)]
    yt = np.concatenate(outs, axis=0)          # [64, 128, 2048] f16
    out = yt.astype(np.float32).transpose(0, 2, 1).reshape(4, 16, S, D)
    if DEBUG_MU:
        kernel.dbg = [np.asarray(res.results[i]["dbg"]) for i in range(N_CORES)]
    return out.astype(orig_dtype, copy=False)


if __name__ == "__main__":
    rng = np.random.default_rng(0)
    kv = rng.standard_normal((4, 16, S, D)).astype(np.float32)
    out = kernel(kv_cache=kv, rank=64)
    print("kernel ran, out", out.shape, out.dtype)


# revision 8
# speedup vs baseline: 3.4609x; 1.3449x over previous
"""Trainium2 Bass kernel for batched low-rank (rank-64) KV-cache reconstruction.

Problem: for each of 64 matrices X [2048,128] (f32), compute the rank-64
truncated-SVD reconstruction X_r = U_r diag(s_r) V_r^T = X P where P projects
onto the top-64 eigenspace of G = X^T X.

Per NeuronCore (8 matrices, two groups of 4), all-fp16 iterations:
  G = Xh^T Xh  (Xh = fp16(X), f32 PSUM accumulate)
  mu ladder: 4 probe stages of cubic soft-sign iterations (a=2 growth +
    a=1.5 landing steps) reading trace(B_k) as a soft eigen-count to
    root-find mu between lambda_64 and lambda_65 of G.
  final: 12-step sign iteration (optimized cubic coefficient schedule with
    periodic symmetric "pair" steps); P = (sign+I)/2 emitted by the last
    write-back.
  recon: Y^T = P @ Xh^T via fp16 matmuls (XT loaded by DMA-transpose XBAR).

The cubic step is computed as Ct = I - (b/a) B^2; B' = a * (B @ Ct), folding
`a` into the PSUM->SBUF write-back scale so one identity constant serves all
steps. Probes read trace(B@C) = sum(B*Ct)*a (B, Ct symmetric), skipping the
last matmul. The two groups run as interleaved instruction streams with a
skew of a few steps, so each group's serial DVE/ACT chain (Ct / write-back /
mu update) hides under the other group's PE matmuls.
"""

from contextlib import ExitStack

import numpy as np

import concourse.bass as bass
import concourse.tile as tile
from concourse import bacc, mybir
from concourse.bass_utils import run_bass_kernel_spmd
from concourse.masks import make_identity

F32 = mybir.dt.float32
F16 = mybir.dt.float16
AF = mybir.ActivationFunctionType
OP = mybir.AluOpType
ts = bass.ts

N_CORES = 8
M_PER_CORE = 8
NG = 2               # matrix groups per core
GM = 4               # matrices per group
S, D = 2048, 128
NT = S // 128        # 16 row chunks
SKEW = 3             # instruction-stream skew between the two groups (ticks)

R = 1250.0
OFF = -41.37

GROW = (2.0, 1.0)
LAND = (1.5, 0.5)
# ladder stages: (n_growth, n_land, gain, clamp)
LADDER = [
    (4, 1, 7.75, 28.0),
    (6, 1, 6.0, 8.0),
    (7, 2, 2.8, 4.0),
    (10, 2, 0.55, 1.0),
]
# final sign schedule (a, b), all fp16
FINAL = [
    (3.25, 3.10818),
    (2.59, 1.575008),
    (2.59, 1.571647),
    (2.59, 1.573557),
    (2.58, 1.553639),
    (2.56, 1.519041),
    (2.52, 1.450415),
    (2.4, 1.250828),
    (2.14, 0.887286),
    (1.78, 0.510408),
    (1.206349, 0.259768),
    (1.5, 0.5),
]


def _pair_steps(k):
    """Every 3rd step runs as a symmetric pair step (never the last)."""
    out = {i for i in range(k) if (i + 1) % 3 == 0}
    out.discard(k - 1)
    return out


class Ctx:
    pass


def _kernel_body(tc, nc, ctx, x, y):
    c = Ctx()
    consts = ctx.enter_context(tc.tile_pool(name="consts", bufs=1))
    c.ident = consts.tile([128, 128], F32, tag="ident", name="ident")
    make_identity(nc, c.ident[:])
    c.iOverR = consts.tile([128, 128], F32, tag="iOverR", name="iOverR")
    nc.vector.tensor_scalar_mul(c.iOverR[:], c.ident[:], 1.0 / R)
    c.I_rep = consts.tile([128, 512], F32, tag="I_rep", name="I_rep")
    for b in range(GM):
        nc.gpsimd.tensor_copy(c.I_rep[:, ts(b, 128)], c.ident[:])
    c.I05_rep = consts.tile([128, 512], F32, tag="I05_rep", name="I05_rep")
    nc.vector.tensor_scalar_mul(c.I05_rep[:], c.I_rep[:], 0.5)
    c.ones_col = consts.tile([128, 1], F32, tag="ones_col", name="ones_col")
    nc.vector.memset(c.ones_col[:], 1.0)
    c.ones_row = consts.tile([1, 128], F32, tag="ones_row", name="ones_row")
    nc.vector.memset(c.ones_row[:], 1.0)
    c.trpack = [consts.tile([128, GM], F32, tag=f"trpack{g}", name=f"trpack{g}") for g in range(NG)]
    c.mu_row = [consts.tile([1, GM], F32, tag=f"mu_row{g}", name=f"mu_row{g}") for g in range(NG)]
    c.step_row = [consts.tile([1, GM], F32, tag=f"step{g}", name=f"step{g}") for g in range(NG)]
    c.MU = [consts.tile([128, GM], F32, tag=f"MU{g}", name=f"MU{g}") for g in range(NG)]

    pools = Ctx()
    xt_pool = ctx.enter_context(tc.tile_pool(name="xt", bufs=1))
    xp_pool = ctx.enter_context(tc.tile_pool(name="xp", bufs=1))
    gpool = ctx.enter_context(tc.tile_pool(name="G", bufs=1))
    bpool = ctx.enter_context(tc.tile_pool(name="B", bufs=1))
    pools.ct = ctx.enter_context(tc.tile_pool(name="ct", bufs=2))
    pools.scr = ctx.enter_context(tc.tile_pool(name="scr", bufs=2))
    mpool = ctx.enter_context(tc.tile_pool(name="muI", bufs=1))
    opool = ctx.enter_context(tc.tile_pool(name="osb", bufs=3))
    pools.p2 = ctx.enter_context(tc.tile_pool(name="p2", bufs=1, space="PSUM"))
    pools.p3 = ctx.enter_context(tc.tile_pool(name="p3", bufs=1, space="PSUM"))
    pools.pmisc = ctx.enter_context(tc.tile_pool(name="pmisc", bufs=1, space="PSUM"))

    XT = [xt_pool.tile([128, S], F16, tag=f"XT{m}", name=f"XT{m}") for m in range(M_PER_CORE)]
    XP = [xp_pool.tile([128, S], F16, tag=f"XP{m}", name=f"XP{m}", bufs=1)
          for m in range(M_PER_CORE)]
    G_all = [gpool.tile([128, 512], F32, tag=f"G{g}", name=f"G{g}") for g in range(NG)]
    B16 = [bpool.tile([128, 512], F16, tag=f"B16_{g}", name=f"B16_{g}") for g in range(NG)]
    P16 = [bpool.tile([128, 512], F16, tag=f"P16_{g}", name=f"P16_{g}") for g in range(NG)]
    muI = [mpool.tile([128, 512], F32, tag=f"muI{g}", name=f"muI{g}") for g in range(NG)]

    def trace_cols(g, in0_ap, in1_ap):
        scr = pools.scr.tile([128, 512], F32, tag=f"scr{g}", name=f"scr{g}", bufs=2)
        nc.vector.tensor_tensor(scr[:], in0_ap, in1_ap, op=OP.mult)
        nc.vector.tensor_reduce(
            c.trpack[g][:, :], scr[:].rearrange("p (b d) -> p b d", d=128),
            axis=mybir.AxisListType.X, op=OP.add,
        )

    def mu_chain(g, scale, offset=None, clamp=None):
        """trpack -> mu_row update -> MU broadcast (one boundary tick)."""
        ps_tr = pools.pmisc.tile([1, GM], F32, tag=f"pstr{g}", name=f"pstr{g}", bufs=1)
        nc.tensor.matmul(ps_tr[:], c.ones_col[:], c.trpack[g][:], start=True, stop=True)
        if offset is not None:
            nc.vector.tensor_scalar(
                out=c.mu_row[g][:], in0=ps_tr[:], scalar1=scale, scalar2=offset,
                op0=OP.mult, op1=OP.add,
            )
        else:
            nc.vector.tensor_scalar(
                out=c.step_row[g][:], in0=ps_tr[:], scalar1=scale, scalar2=clamp,
                op0=OP.mult, op1=OP.min,
            )
            nc.vector.tensor_scalar_max(c.step_row[g][:], c.step_row[g][:], -clamp)
            nc.vector.tensor_tensor(
                c.mu_row[g][:], c.mu_row[g][:], c.step_row[g][:], op=OP.add
            )
        ps_mu = pools.pmisc.tile([128, GM], F32, tag=f"psmu{g}", name=f"psmu{g}", bufs=1)
        nc.tensor.matmul(ps_mu[:], c.ones_row[:], c.mu_row[g][:], start=True, stop=True)
        nc.scalar.copy(c.MU[g][:], ps_mu[:])

    def b0_tick(g):
        """MuI blocks (ACT, per-partition scale) + batched B0 STT (DVE)."""
        for j in range(GM):
            nc.scalar.activation(
                muI[g][:, ts(j, 128)], c.iOverR[:], AF.Copy,
                scale=c.MU[g][:, j : j + 1],
            )
        nc.vector.scalar_tensor_tensor(
            out=B16[g][:], in0=G_all[g][:], scalar=1.0 / R, in1=muI[g][:],
            op0=OP.mult, op1=OP.subtract,
        )

    def cubic_step(g, a, b, is_pair, last_trace=False, emit_P=False):
        B = B16[g][:]
        ps2 = pools.p2.tile([128, 512], F32, tag=f"ps2g{g}", name=f"ps2g{g}", bufs=1)
        for j in range(GM):
            nc.tensor.matmul(
                ps2[:, ts(j, 128)], B[:, ts(j, 128)], B[:, ts(j, 128)],
                start=True, stop=True,
            )
        Ct = pools.ct.tile([128, 512], F16, tag=f"ct{g}", name=f"ct{g}", bufs=2)
        nc.vector.scalar_tensor_tensor(
            out=Ct[:], in0=ps2[:], scalar=-(b / a), in1=c.I_rep[:],
            op0=OP.mult, op1=OP.add,
        )
        if last_trace:
            trace_cols(g, B, Ct[:])
            return
        ps3 = pools.p3.tile([128, 512], F32, tag=f"ps3g{g}", name=f"ps3g{g}", bufs=1)
        for j in range(GM):
            nc.tensor.matmul(
                ps3[:, ts(j, 128)], B[:, ts(j, 128)], Ct[:, ts(j, 128)],
                start=True, stop=not is_pair,
            )
            if is_pair:
                # close each block's accumulation group before the next opens:
                # only one PSUM accumulation group may be open at a time
                nc.tensor.matmul(
                    ps3[:, ts(j, 128)], Ct[:, ts(j, 128)], B[:, ts(j, 128)],
                    start=False, stop=True, skip_group_check=True,
                )
        scale = a / 2.0 if is_pair else a
        if emit_P:
            nc.vector.scalar_tensor_tensor(
                out=P16[g][:], in0=ps3[:], scalar=0.5 * scale, in1=c.I05_rep[:],
                op0=OP.mult, op1=OP.add,
            )
        else:
            nc.scalar.activation(B, ps3[:], AF.Copy, scale=scale)

    def group_stream(g):
        """Yields once per 'tick'; emits that tick's instructions for group g."""
        # Gram + seed
        psG = pools.p2.tile([128, 512], F32, tag=f"ps2g{g}", name=f"psG{g}", bufs=1)
        for j in range(GM):
            m = g * GM + j
            for t in range(NT):
                nc.tensor.matmul(
                    psG[:, ts(j, 128)], XP[m][:, ts(t, 128)], XP[m][:, ts(t, 128)],
                    start=(t == 0), stop=(t == NT - 1),
                )
            yield
        nc.scalar.copy(G_all[g][:], psG[:])
        trace_cols(g, G_all[g][:], c.I_rep[:])
        mu_chain(g, 1.0 / 128.0, offset=OFF)
        yield
        # ladder
        for (n_grow, n_land, gain, clamp) in LADDER:
            coeffs = [GROW] * n_grow + [LAND] * n_land
            k = len(coeffs)
            pair = _pair_steps(k)
            b0_tick(g)
            yield
            for i, (a, b) in enumerate(coeffs):
                cubic_step(g, a, b, i in pair, last_trace=(i == k - 1))
                yield
            a_last = coeffs[-1][0]
            mu_chain(g, gain * a_last, clamp=clamp)
            yield
        # final
        k = len(FINAL)
        pair = _pair_steps(k)
        b0_tick(g)
        yield
        for i, (a, b) in enumerate(FINAL):
            cubic_step(g, a, b, i in pair, emit_P=(i == k - 1))
            yield
        # recon
        for j in range(GM):
            m = g * GM + j
            for ch in range(S // 512):
                psO = pools.p3.tile([128, 512], F32, tag=f"ps3g{g}", name=f"psO{g}", bufs=1)
                nc.tensor.matmul(
                    psO[:], P16[g][:, ts(j, 128)], XT[m][:, ts(ch, 512)],
                    start=True, stop=True,
                )
                osb = opool.tile([128, 512], F16, tag=f"o{g}_{ch % 2}", name=f"o{g}", bufs=2)
                if ch % 2 == 0:
                    nc.scalar.copy(osb[:], psO[:])
                else:
                    nc.vector.tensor_copy(osb[:], psO[:])
                eng = nc.sync if m % 2 == 0 else nc.scalar
                eng.dma_start(y[m, :, ts(ch, 512)], osb[:])
                yield

    # ---- input DMAs ----
    # packed row layout: XP[m][p, t*128+d] = X[m][p*16+t, d]  (4KB runs; any
    # row permutation is valid for the Gram accumulation)
    for m in range(M_PER_CORE):
        src = x[m].rearrange("(p t) d -> p t d", t=NT)
        dst = XP[m][:].rearrange("p (t d) -> p t d", d=128)
        eng = nc.sync if m % 2 == 0 else nc.scalar
        eng.dma_start(dst, src)
    # XT via DMA-transpose XBAR (needed only at recon; fills during ladder)
    for m in range(M_PER_CORE):
        nc.sync.dma_start(XT[m][:], x[m], transpose=True)

    # ---- interleave the two group streams with a skew ----
    streams = [group_stream(g) for g in range(NG)]
    done = [False] * NG
    tick = 0
    while not all(done):
        for g in range(NG):
            if g == 1 and tick < SKEW:
                continue
            if not done[g]:
                try:
                    next(streams[g])
                except StopIteration:
                    done[g] = True
        tick += 1


_NC_CACHE = {}


def _build_program():
    if "nc" in _NC_CACHE:
        return _NC_CACHE["nc"]
    nc = bacc.Bacc(
        "TRN2",
        target_bir_lowering=False,
        debug=False,
        enable_asserts=True,
        num_devices=N_CORES,
    )
    x = nc.dram_tensor("x", [M_PER_CORE, S, D], F16, kind="ExternalInput").ap()
    y = nc.dram_tensor("y", [M_PER_CORE, D, S], F16, kind="ExternalOutput").ap()
    with tile.TileContext(nc) as tc:
        with ExitStack() as ctx:
            _kernel_body(tc, nc, ctx, x, y)
    nc.compile()
    _NC_CACHE["nc"] = nc
    return nc


def kernel(kv_cache, rank, **_ignored):
    kv = np.asarray(kv_cache)
    assert kv.shape == (4, 16, S, D), kv.shape
    assert int(rank) == 64, rank
    orig_dtype = kv.dtype
    xs = np.ascontiguousarray(kv.reshape(-1, S, D)).astype(np.float16)

    nc = _build_program()
    in_maps = [
        {"x": xs[i * M_PER_CORE : (i + 1) * M_PER_CORE]} for i in range(N_CORES)
    ]
    res = run_bass_kernel_spmd(nc, in_maps, list(range(N_CORES)))
    outs = [np.asarray(res.results[i]["y"]) for i in range(N_CORES)]
    yt = np.concatenate(outs, axis=0)          # [64, 128, 2048] f16
    out = yt.astype(np.float32).transpose(0, 2, 1).reshape(4, 16, S, D)
    return out.astype(orig_dtype, copy=False)


if __name__ == "__main__":
    rng = np.random.default_rng(0)
    kv = rng.standard_normal((4, 16, S, D)).astype(np.float32)
    out = kernel(kv_cache=kv, rank=64)
    print("kernel ran, out", out.shape, out.dtype)


# revision 10
# speedup vs baseline: 3.7719x; 1.0899x over previous
"""Trainium2 Bass kernel for batched low-rank (rank-64) KV-cache reconstruction.

Problem: for each of 64 matrices X [2048,128] (f32), compute the rank-64
truncated-SVD reconstruction X_r = U_r diag(s_r) V_r^T = X P where P projects
onto the top-64 eigenspace of G = X^T X.

Per NeuronCore (8 matrices, two groups of 4), all-fp16 iterations:
  G = Xh^T Xh  (Xh = fp16(X), f32 PSUM accumulate)
  mu ladder: 4 probe stages of cubic soft-sign iterations (a=2 growth +
    a=1.5 landing steps) reading trace(B_k) as a soft eigen-count to
    root-find mu between lambda_64 and lambda_65 of G.
  final: 12-step sign iteration (optimized cubic coefficient schedule with
    periodic symmetric "pair" steps); P = (sign+I)/2 emitted by the last
    write-back.
  recon: Y^T = P @ Xh^T via fp16 matmuls (XT loaded by DMA-transpose XBAR).

The cubic step is computed as Ct = I - (b/a) B^2; B' = a * (B @ Ct), folding
`a` into the PSUM->SBUF write-back scale so one identity constant serves all
steps. Probes read trace(B@C) = sum(B*Ct)*a (B, Ct symmetric), skipping the
last matmul. The two groups run as interleaved instruction streams with a
skew of a few steps, so each group's serial DVE/ACT chain (Ct / write-back /
mu update) hides under the other group's PE matmuls.
"""

from contextlib import ExitStack

import numpy as np

import concourse.bass as bass
import concourse.tile as tile
from concourse import bacc, mybir
from concourse.bass_utils import run_bass_kernel_spmd
from concourse.masks import make_identity

F32 = mybir.dt.float32
F16 = mybir.dt.float16
AF = mybir.ActivationFunctionType
OP = mybir.AluOpType
ts = bass.ts

N_CORES = 8
M_PER_CORE = 8
NG = 2               # matrix groups per core
GM = 4               # matrices per group
S, D = 2048, 128
NT = S // 128        # 16 row chunks
SKEW = 3             # instruction-stream skew between the two groups (ticks)

R = 1250.0
OFF = -41.37

GROW = (2.0, 1.0)
LAND = (1.5, 0.5)
# ladder stages: (n_growth, n_land, gain, clamp)
LADDER = [
    (4, 1, 7.75, 28.0),
    (6, 1, 6.0, 8.0),
    (7, 2, 2.8, 4.0),
    (9, 2, 1.0, 1.2),
]
# final sign schedule (a, b), all fp16
FINAL = [
    (3.25, 3.10818),
    (2.59, 1.575008),
    (2.59, 1.571647),
    (2.59, 1.573557),
    (2.58, 1.553639),
    (2.56, 1.519041),
    (2.52, 1.450415),
    (2.4, 1.250828),
    (2.14, 0.887286),
    (1.427341, 0.407817),
    (1.500676, 0.500097),
]


def _pair_steps(k):
    """Every 4th step runs as a symmetric pair step (never the last)."""
    out = {i for i in range(k) if (i + 1) % 4 == 0}
    out.discard(k - 1)
    return out


class Ctx:
    pass


def _kernel_body(tc, nc, ctx, x, y):
    c = Ctx()
    consts = ctx.enter_context(tc.tile_pool(name="consts", bufs=1))
    c.ident = consts.tile([128, 128], F32, tag="ident", name="ident")
    make_identity(nc, c.ident[:])
    c.I_rep = consts.tile([128, 512], F32, tag="I_rep", name="I_rep")
    for b in range(GM):
        nc.gpsimd.tensor_copy(c.I_rep[:, ts(b, 128)], c.ident[:])
    c.I05_rep = consts.tile([128, 512], F32, tag="I05_rep", name="I05_rep")
    nc.vector.tensor_scalar_mul(c.I05_rep[:], c.I_rep[:], 0.5)
    c.ones_col = consts.tile([128, 1], F32, tag="ones_col", name="ones_col")
    nc.vector.memset(c.ones_col[:], 1.0)
    c.ones_row = consts.tile([1, 128], F32, tag="ones_row", name="ones_row")
    nc.vector.memset(c.ones_row[:], 1.0)
    c.neg_row = consts.tile([1, 128], F32, tag="neg_row", name="neg_row")
    nc.vector.memset(c.neg_row[:], -1.0)
    c.trpack = [consts.tile([128, GM], F32, tag=f"trpack{g}", name=f"trpack{g}") for g in range(NG)]
    c.mu_row = [consts.tile([1, GM], F32, tag=f"mu_row{g}", name=f"mu_row{g}") for g in range(NG)]
    c.step_row = [consts.tile([1, GM], F32, tag=f"step{g}", name=f"step{g}") for g in range(NG)]

    pools = Ctx()
    xt_pool = ctx.enter_context(tc.tile_pool(name="xt", bufs=1))
    xp_pool = ctx.enter_context(tc.tile_pool(name="xp", bufs=1))
    gpool = ctx.enter_context(tc.tile_pool(name="G", bufs=1))
    bpool = ctx.enter_context(tc.tile_pool(name="B", bufs=1))
    pools.ct = ctx.enter_context(tc.tile_pool(name="ct", bufs=2))
    pools.scr = ctx.enter_context(tc.tile_pool(name="scr", bufs=2))
    opool = ctx.enter_context(tc.tile_pool(name="osb", bufs=3))
    pools.p2 = ctx.enter_context(tc.tile_pool(name="p2", bufs=1, space="PSUM"))
    pools.p3 = ctx.enter_context(tc.tile_pool(name="p3", bufs=1, space="PSUM"))
    pools.pmisc = ctx.enter_context(tc.tile_pool(name="pmisc", bufs=1, space="PSUM"))

    XT = [xt_pool.tile([128, S], F16, tag=f"XT{m}", name=f"XT{m}") for m in range(M_PER_CORE)]
    XP = [xp_pool.tile([128, S], F16, tag=f"XP{m}", name=f"XP{m}", bufs=1)
          for m in range(M_PER_CORE)]
    G_all = [gpool.tile([128, 512], F32, tag=f"G{g}", name=f"G{g}") for g in range(NG)]
    B16 = [bpool.tile([128, 512], F16, tag=f"B16_{g}", name=f"B16_{g}") for g in range(NG)]
    P16 = [bpool.tile([128, 512], F16, tag=f"P16_{g}", name=f"P16_{g}") for g in range(NG)]

    def trace_cols(g, in0_ap, in1_ap):
        scr = pools.scr.tile([128, 512], F32, tag=f"scr{g}", name=f"scr{g}", bufs=2)
        nc.vector.tensor_tensor(scr[:], in0_ap, in1_ap, op=OP.mult)
        nc.vector.tensor_reduce(
            c.trpack[g][:, :], scr[:].rearrange("p (b d) -> p b d", d=128),
            axis=mybir.AxisListType.X, op=OP.add,
        )

    def mu_chain(g, scale, offset=None, clamp=None):
        """trpack -> mu_row update -> MU broadcast (one boundary tick)."""
        ps_tr = pools.pmisc.tile([1, GM], F32, tag=f"pstr{g}", name=f"pstr{g}", bufs=1)
        nc.tensor.matmul(ps_tr[:], c.ones_col[:], c.trpack[g][:], start=True, stop=True)
        if offset is not None:
            nc.vector.tensor_scalar(
                out=c.mu_row[g][:], in0=ps_tr[:], scalar1=scale, scalar2=offset,
                op0=OP.mult, op1=OP.add,
            )
        else:
            nc.vector.tensor_scalar(
                out=c.step_row[g][:], in0=ps_tr[:], scalar1=scale, scalar2=clamp,
                op0=OP.mult, op1=OP.min,
            )
            nc.vector.tensor_scalar_max(c.step_row[g][:], c.step_row[g][:], -clamp)
            nc.vector.tensor_tensor(
                c.mu_row[g][:], c.mu_row[g][:], c.step_row[g][:], op=OP.add
            )
        # broadcast NEGATED nu so b0_tick's STT can use a plain multiply
        ps_mu = pools.pmisc.tile([128, GM], F32, tag=f"psmu{g}", name=f"psmu{g}", bufs=1)
        nc.tensor.matmul(ps_mu[:], c.neg_row[:], c.mu_row[g][:], start=True, stop=True)
        return ps_mu

    def b0_tick(g, ps_mu):
        """B0 = G/R - nu*I per block: one STT per block, nu broadcast from PSUM."""
        for j in range(GM):
            nc.vector.scalar_tensor_tensor(
                out=B16[g][:, ts(j, 128)], in0=c.ident[:],
                scalar=ps_mu[:, j : j + 1], op0=OP.mult,
                in1=G_all[g][:, ts(j, 128)], op1=OP.add,
            )

    def cubic_step(g, a, b, is_pair, last_trace=False, emit_P=False):
        B = B16[g][:]
        ps2 = pools.p2.tile([128, 512], F32, tag=f"ps2g{g}", name=f"ps2g{g}", bufs=1)
        for j in range(GM):
            nc.tensor.matmul(
                ps2[:, ts(j, 128)], B[:, ts(j, 128)], B[:, ts(j, 128)],
                start=True, stop=True,
            )
        Ct = pools.ct.tile([128, 512], F16, tag=f"ct{g}", name=f"ct{g}", bufs=2)
        nc.vector.scalar_tensor_tensor(
            out=Ct[:], in0=ps2[:], scalar=-(b / a), in1=c.I_rep[:],
            op0=OP.mult, op1=OP.add,
        )
        if last_trace:
            trace_cols(g, B, Ct[:])
            return
        ps3 = pools.p3.tile([128, 512], F32, tag=f"ps3g{g}", name=f"ps3g{g}", bufs=1)
        for j in range(GM):
            nc.tensor.matmul(
                ps3[:, ts(j, 128)], B[:, ts(j, 128)], Ct[:, ts(j, 128)],
                start=True, stop=not is_pair,
            )
            if is_pair:
                # close each block's accumulation group before the next opens:
                # only one PSUM accumulation group may be open at a time
                nc.tensor.matmul(
                    ps3[:, ts(j, 128)], Ct[:, ts(j, 128)], B[:, ts(j, 128)],
                    start=False, stop=True, skip_group_check=True,
                )
        scale = a / 2.0 if is_pair else a
        if emit_P:
            nc.vector.scalar_tensor_tensor(
                out=P16[g][:], in0=ps3[:], scalar=0.5 * scale, in1=c.I05_rep[:],
                op0=OP.mult, op1=OP.add,
            )
        else:
            nc.scalar.activation(B, ps3[:], AF.Copy, scale=scale)

    def group_stream(g):
        """Yields once per 'tick'; emits that tick's instructions for group g."""
        # Gram + seed
        psG = pools.p2.tile([128, 512], F32, tag=f"ps2g{g}", name=f"psG{g}", bufs=1)
        for j in range(GM):
            m = g * GM + j
            for t in range(NT):
                nc.tensor.matmul(
                    psG[:, ts(j, 128)], XP[m][:, ts(t, 128)], XP[m][:, ts(t, 128)],
                    start=(t == 0), stop=(t == NT - 1),
                )
            yield
        nc.scalar.activation(G_all[g][:], psG[:], AF.Copy, scale=1.0 / R)
        trace_cols(g, G_all[g][:], c.I_rep[:])
        ps_mu = mu_chain(g, 1.0 / 128.0, offset=OFF / R)
        yield
        # ladder
        for (n_grow, n_land, gain, clamp) in LADDER:
            coeffs = [GROW] * n_grow + [LAND] * n_land
            k = len(coeffs)
            pair = _pair_steps(k)
            b0_tick(g, ps_mu)
            yield
            for i, (a, b) in enumerate(coeffs):
                cubic_step(g, a, b, i in pair, last_trace=(i == k - 1))
                yield
            a_last = coeffs[-1][0]
            ps_mu = mu_chain(g, gain * a_last / R, clamp=clamp / R)
            yield
        # final
        k = len(FINAL)
        pair = _pair_steps(k)
        b0_tick(g, ps_mu)
        yield
        for i, (a, b) in enumerate(FINAL):
            cubic_step(g, a, b, i in pair, emit_P=(i == k - 1))
            yield
        # recon
        for j in range(GM):
            m = g * GM + j
            for ch in range(S // 512):
                psO = pools.p3.tile([128, 512], F32, tag=f"ps3g{g}", name=f"psO{g}", bufs=1)
                nc.tensor.matmul(
                    psO[:], P16[g][:, ts(j, 128)], XT[m][:, ts(ch, 512)],
                    start=True, stop=True,
                )
                osb = opool.tile([128, 512], F16, tag=f"o{g}_{ch % 2}", name=f"o{g}", bufs=2)
                if ch % 2 == 0:
                    nc.scalar.copy(osb[:], psO[:])
                else:
                    nc.vector.tensor_copy(osb[:], psO[:])
                eng = nc.sync if (m + ch) % 2 == 0 else nc.scalar
                eng.dma_start(y[m, :, ts(ch, 512)], osb[:])
                yield

    # ---- input DMAs ----
    # packed row layout: XP[m][p, t*128+d] = X[m][p*16+t, d]  (4KB runs; any
    # row permutation is valid for the Gram accumulation)
    for m in range(M_PER_CORE):
        src = x[m].rearrange("(p t) d -> p t d", t=NT)
        dst = XP[m][:].rearrange("p (t d) -> p t d", d=128)
        eng = nc.sync if m % 2 == 0 else nc.scalar
        eng.dma_start(dst, src)
    # XT via DMA-transpose XBAR (needed only at recon; fills during ladder)
    for m in range(M_PER_CORE):
        nc.sync.dma_start(XT[m][:], x[m], transpose=True)

    # ---- interleave the two group streams with a skew ----
    streams = [group_stream(g) for g in range(NG)]
    done = [False] * NG
    tick = 0
    while not all(done):
        for g in (1, 0):
            if g == 1 and tick < SKEW:
                continue
            if not done[g]:
                try:
                    next(streams[g])
                except StopIteration:
                    done[g] = True
        tick += 1


_NC_CACHE = {}


def _build_program():
    if "nc" in _NC_CACHE:
        return _NC_CACHE["nc"]
    nc = bacc.Bacc(
        "TRN2",
        target_bir_lowering=False,
        debug=False,
        enable_asserts=True,
        num_devices=N_CORES,
    )
    x = nc.dram_tensor("x", [M_PER_CORE, S, D], F16, kind="ExternalInput").ap()
    y = nc.dram_tensor("y", [M_PER_CORE, D, S], F16, kind="ExternalOutput").ap()
    with tile.TileContext(nc) as tc:
        with ExitStack() as ctx:
            _kernel_body(tc, nc, ctx, x, y)
    nc.compile()
    _NC_CACHE["nc"] = nc
    return nc


def kernel(kv_cache, rank, **_ignored):
    kv = np.asarray(kv_cache)
    assert kv.shape == (4, 16, S, D), kv.shape
    assert int(rank) == 64, rank
    orig_dtype = kv.dtype
    xs = np.ascontiguousarray(kv.reshape(-1, S, D)).astype(np.float16)

    nc = _build_program()
    in_maps = [
        {"x": xs[i * M_PER_CORE : (i + 1) * M_PER_CORE]} for i in range(N_CORES)
    ]
    res = run_bass_kernel_spmd(nc, in_maps, list(range(N_CORES)))
    outs = [np.asarray(res.results[i]["y"]) for i in range(N_CORES)]
    yt = np.concatenate(outs, axis=0)          # [64, 128, 2048] f16
    out = yt.astype(np.float32).transpose(0, 2, 1).reshape(4, 16, S, D)
    return out.astype(orig_dtype, copy=False)


if __name__ == "__main__":
    rng = np.random.default_rng(0)
    kv = rng.standard_normal((4, 16, S, D)).astype(np.float32)
    out = kernel(kv_cache=kv, rank=64)
    print("kernel ran, out", out.shape, out.dtype)
